# revision 6
# baseline (speedup 1.0000x reference)
"""Causal self-attention Trainium2 kernel (8 NeuronCores).

Sharding: data-parallel over batch (2) x tensor-parallel over head groups
(12 heads -> 4 groups of 3). Core c handles batch c//4, head group c%4.
Each core computes its partial projection output; the host sums the 4
partials per batch (the TP reduce folded into the output gather).

Host pre-stages per core (all bf16):
  xT [C, T]      x[b] transposed on host (no on-device transposes)
  wqk [C, 384]   columns [q0|q1|k0|k1|q2|k2] (64 each)
  wv  [C, 192]   columns [v0|v1|v2]
  wp  [192, C]   Wproj rows for the 3 local heads

Per-core dataflow (T=2048, C=768, local heads h0..h2, HD=64):
  qkT tiles [128,T]: t0=[q0|q1], t1=[k0|k1], t2=[q2|k2] = Wqk.T @ xT
  v_sb natural [128, 3*16*65] (ones col per t-tile) from xT-stationary
    matmuls: psum[t,192] = sum_cc xT[cc,t-tile].T @ Wv[cc]
  per head: S^T_j [128 keys, w] = k_h[:,jblk].T-rows @ q_h (bf16 K=64,
    h0/h1 packed in disjoint PE row groups; h2 split over both row
    groups by j parity using a partition-swapped dup tile)
  P = exp(S^T/8) (ACT -> bf16), causal diag masked on GpSimd
  yq [65,512] per query chunk: += [v_h|1].T @ P_j; denom row 64
  normalize via f32r ones-outer-product bcast + DVE recip/mul
  out[t-tile, C] = yT_a.T-slices @ wp[:128] + yT_b-slices @ wp[128:]
    (K=64 leg paired across adjacent t-tiles on disjoint row groups)
"""

import functools

import numpy as np
import ml_dtypes

import concourse.bass as bass
import concourse.mybir as mybir
import concourse.tile as tile
from concourse import bacc
from concourse.bass_utils import run_bass_kernel_spmd
from concourse.masks import make_upper_triangular

P = 128
B, T, C = 2, 2048, 768
NH, HD = 12, 64
HPG = 3            # heads per core
NCC = C // P       # 6 contraction tiles
NT = T // P        # 16 t-tiles
QKW = 6 * HD       # 384 q+k channels
VW = HPG * HD      # 192 v channels
F32 = mybir.dt.float32
F32R = mybir.dt.float32r
BF16 = mybir.dt.bfloat16
BF16NP = ml_dtypes.bfloat16

# causal exp-buffer layout: key-tile j at offset OFFS[j], width 2048-128*j
OFFS = []
_o = 0
for _j in range(NT):
    OFFS.append(_o)
    _o += T - P * _j
EXPW = _o  # 17408

LAST_RESULT = None


def _emit(nc, tc, xt_d, wqk_d, wv_d, wp_d, out_d):
    from contextlib import ExitStack

    ctx = ExitStack()
    with ctx:
        const = ctx.enter_context(tc.tile_pool(name="const", bufs=1))
        tri = const.tile([P, P], BF16)
        make_upper_triangular(nc, tri[:], val=1.0, diag=True)
        ones_f = const.tile([P, HD], F32)
        nc.any.memset(ones_f[:], 1.0)
        ones_r = const.tile([P, HD], F32R)
        nc.vector.tensor_copy(out=ones_r[:], in_=ones_f[:])

        # ---------------- weight + x DMAs ----------------
        w_pool = ctx.enter_context(tc.tile_pool(name="w", bufs=1))
        wqk_sb = w_pool.tile([P, NCC * QKW], BF16, tag="wqk")
        wv_sb = w_pool.tile([P, NCC * VW], BF16, tag="wv")
        wp_a = w_pool.tile([P, C], BF16, tag="wpa")
        wp_b = w_pool.tile([P, C], BF16, tag="wpb")  # rows 0:64 and 64:128 both = wp[128:192]
        for cc in range(NCC):
            nc.sync.dma_start(
                wqk_sb[:, cc * QKW : (cc + 1) * QKW],
                wqk_d[cc * P : (cc + 1) * P, :],
            )
            nc.sync.dma_start(
                wv_sb[:, cc * VW : (cc + 1) * VW],
                wv_d[cc * P : (cc + 1) * P, :],
            )
        nc.sync.dma_start(wp_a[:], wp_d[0:P, :])
        nc.sync.dma_start(wp_b[0:HD, :], wp_d[P : P + HD, :])
        nc.sync.dma_start(wp_b[HD:P, :], wp_d[P : P + HD, :])

        x_pool = ctx.enter_context(tc.tile_pool(name="x", bufs=1))
        xt_sb = x_pool.tile([P, NCC * T], BF16, tag="xt")
        for cc in range(NCC):
            nc.sync.dma_start(
                xt_sb[:, cc * T : (cc + 1) * T], xt_d[cc * P : (cc + 1) * P, :]
            )

        qk_pool = ctx.enter_context(tc.tile_pool(name="qk", bufs=1))
        qk = [qk_pool.tile([P, T], BF16, tag=f"qk{i}", name=f"qk{i}") for i in range(3)]
        dup = qk_pool.tile([P, T], BF16, tag="dup", name="dup")  # [k2 | q2]

        v_pool = ctx.enter_context(tc.tile_pool(name="v", bufs=1))
        v_sb = v_pool.tile([P, HPG * NT * (HD + 1)], BF16, tag="v")
        nc.vector.memset(v_sb[:], 1.0)  # ones cols; data cols overwritten
        v4 = v_sb[:].rearrange("p (h t d) -> p h t d", h=HPG, t=NT)

        exp_pool = ctx.enter_context(tc.tile_pool(name="exp", bufs=1))
        ex = [
            exp_pool.tile([P, EXPW], BF16, tag=f"ex{h}", name=f"ex{h}")
            for h in range(HPG)
        ]

        y_pool = ctx.enter_context(tc.tile_pool(name="y", bufs=1))
        yT_a = y_pool.tile([P, T], BF16, tag="ya")   # h0 rows 0:64, h1 64:128
        yT_b = y_pool.tile([P, T], BF16, tag="yb")   # h2 rows 0:64, dup 64:128
        nrm_pool = ctx.enter_context(tc.tile_pool(name="nrm", bufs=2))
        out_pool = ctx.enter_context(tc.tile_pool(name="outp", bufs=3))

        # PSUM pools: ps_qkv (4 banks) + st (4 banks) early; after qkv
        # closes: v (1) + yq (2) + bc (1) + st (4); all closed before proj.
        st_cm = tc.tile_pool(name="ps_st", bufs=2, space="PSUM")
        st_pool = st_cm.__enter__()
        ps_qkv_cm = tc.tile_pool(name="ps_qkv", bufs=4, space="PSUM")
        ps_qkv = ps_qkv_cm.__enter__()

        def qkv_tile(i):
            """qk[i] = (wqk cols i*128..) .T @ xT, all 4 chunks."""
            pst = [ps_qkv.tile([P, 512], F32, tag="qkv", name="qkv") for _ in range(4)]
            for cc in range(NCC):
                lhs = wqk_sb[:, cc * QKW + i * P : cc * QKW + (i + 1) * P]
                for tch in range(4):
                    nc.tensor.matmul(
                        pst[tch][:],
                        lhs,
                        xt_sb[:, cc * T + tch * 512 : cc * T + (tch + 1) * 512],
                        start=(cc == 0),
                        stop=(cc == NCC - 1),
                    )
            for tch in range(4):
                nc.vector.tensor_copy(
                    out=qk[i][:, tch * 512 : (tch + 1) * 512], in_=pst[tch][:]
                )

        ps_v_holder = [None]

        def v_tiles(tts):
            """v_sb natural tiles for t-tiles tts via xT-stationary matmuls."""
            for tt in tts:
                psv = ps_v_holder[0].tile([P, VW], F32, tag="vt")
                for cc in range(NCC):
                    nc.tensor.matmul(
                        psv[:],
                        xt_sb[:, cc * T + tt * P : cc * T + (tt + 1) * P],
                        wv_sb[:, cc * VW : (cc + 1) * VW],
                        start=(cc == 0),
                        stop=(cc == NCC - 1),
                    )
                nc.vector.tensor_copy(
                    out=v4[:, :, tt, 0:HD],
                    in_=psv[:].rearrange("p (h d) -> p h d", d=HD),
                )

        # q/k slices per head: (tile, partition offset)
        q_sl = [(0, 0), (0, HD), (2, 0)]
        k_sl = [(1, 0), (1, HD), (2, HD)]

        def qk_j_pair(j):
            """S^T tiles for heads 0,1 at key-tile j + exp + diag mask.
            Slots interleave per 512-chunk so the two PE row groups overlap."""
            w = T - P * j
            tq0 = P * j
            done = 0
            while done < w:
                cw = min(1024, w - done)
                sts = [st_pool.tile([P, 1024], F32, tag="st", name="st") for _ in range(2)]
                for s0 in range(0, cw, 512):
                    sw = min(512, cw - s0)
                    for sl in range(2):
                        qi, qo = q_sl[sl]
                        ki, ko = k_sl[sl]
                        nc.tensor.matmul(
                            sts[sl][:, s0 : s0 + sw],
                            qk[ki][ko : ko + HD, tq0 : tq0 + P],
                            qk[qi][qo : qo + HD, tq0 + done + s0 : tq0 + done + s0 + sw],
                            start=True,
                            stop=True,
                        )
                for sl in range(2):
                    nc.scalar.activation(
                        ex[sl][:, OFFS[j] + done : OFFS[j] + done + cw],
                        sts[sl][:, 0:cw],
                        mybir.ActivationFunctionType.Exp,
                        scale=0.125,
                    )
                done += cw
            for sl in range(2):
                dg = ex[sl][:, OFFS[j] : OFFS[j] + P]
                nc.gpsimd.tensor_mul(out=dg, in0=dg, in1=tri[:])

        def qk_j_h2(j):
            """S^T for head 2 at key-tile j; row group alternates by parity."""
            w = T - P * j
            tq0 = P * j
            if j % 2 == 0:
                kh = dup[0:HD, :]          # k2 at partitions 0:64
                qh = qk[2][0:HD, :]        # q2 natural
            else:
                kh = qk[2][HD:P, :]        # k2 natural at partitions 64:128
                qh = dup[HD:P, :]          # q2 dup
            done = 0
            while done < w:
                cw = min(1024, w - done)
                st = st_pool.tile([P, 1024], F32, tag="st")
                for s0 in range(0, cw, 512):
                    sw = min(512, cw - s0)
                    nc.tensor.matmul(
                        st[:, s0 : s0 + sw],
                        kh[:, tq0 : tq0 + P],
                        qh[:, tq0 + done + s0 : tq0 + done + s0 + sw],
                        start=True,
                        stop=True,
                    )
                nc.scalar.activation(
                    ex[2][:, OFFS[j] + done : OFFS[j] + done + cw],
                    st[:, 0:cw],
                    mybir.ActivationFunctionType.Exp,
                    scale=0.125,
                )
                done += cw
            dg = ex[2][:, OFFS[j] : OFFS[j] + P]
            nc.gpsimd.tensor_mul(out=dg, in0=dg, in1=tri[:])

        yq_holder = [None]
        bc_holder = [None]

        def ydst_of(h):
            return yT_a[0:HD, :] if h == 0 else (
                yT_a[HD:P, :] if h == 1 else yT_b[0:HD, :]
            )

        def av_chunk(h, q):
            """yq[65,512] = sum_j [v|1].T @ P_j for query chunk q, then
            normalize into yT via f32r ones-outer-product broadcast."""
            yq = yq_holder[0].tile([HD + 1, 512], F32, tag="yq")
            for jj in range(4 * q + 4):
                lo = max(512 * q, P * jj)
                hi = 512 * (q + 1)
                so = OFFS[jj] - P * jj
                nc.tensor.matmul(
                    yq[:, lo - 512 * q : hi - 512 * q],
                    v4[:, h, jj, :],
                    ex[h][:, so + lo : so + hi],
                    start=(jj == 0),
                    stop=(jj == 4 * q + 3),
                )
            den = nrm_pool.tile([P, 512], F32R, tag="den")
            nc.vector.tensor_copy(out=den[HD : HD + 1, :], in_=yq[HD : HD + 1, :])
            bc = bc_holder[0].tile([HD, 512], F32, tag="bc")
            nc.tensor.matmul(
                bc[:],
                ones_r[HD : HD + 1, :],
                den[HD : HD + 1, :],
                start=True,
                stop=True,
            )
            bcs = nrm_pool.tile([HD, 512], F32, tag="bcs")
            with nc.allow_low_precision(reason="softmax denom"):
                nc.vector.reciprocal_approx_fast(bcs[:], bc[:])
            nc.vector.tensor_mul(
                out=ydst_of(h)[:, 512 * q : 512 * (q + 1)],
                in0=yq[0:HD, :],
                in1=bcs[:],
            )
            if h == 2:  # duplicate h2 rows to partitions 64:128 for proj pairing
                nc.sync.dma_start(
                    yT_b[HD:P, 512 * q : 512 * (q + 1)],
                    yT_b[0:HD, 512 * q : 512 * (q + 1)],
                )

        # ---------------- emission schedule ----------------
        qkv_tile(0)
        qkv_tile(1)

        ps_v_cm = yq_cm = bc_cm = None
        for j in range(NT):
            qk_j_pair(j)
            if j == 0:
                qkv_tile(2)
                # dup = [k2 | q2] partition swap of qk[2]
                nc.sync.dma_start(dup[0:HD, :], qk[2][HD:P, :])
                nc.sync.dma_start(dup[HD:P, :], qk[2][0:HD, :])
                ps_qkv_cm.__exit__(None, None, None)
                yq_cm = tc.tile_pool(name="ps_yq", bufs=2, space="PSUM")
                yq_holder[0] = yq_cm.__enter__()
                bc_cm = tc.tile_pool(name="ps_bc", bufs=1, space="PSUM")
                bc_holder[0] = bc_cm.__enter__()
                ps_v_cm = tc.tile_pool(name="ps_v", bufs=1, space="PSUM")
                ps_v_holder[0] = ps_v_cm.__enter__()
            elif j in (1, 2, 4, 5):
                base = {1: 0, 2: 4, 4: 8, 5: 12}[j]
                v_tiles(range(base, base + 4))
                if j == 5:
                    ps_v_cm.__exit__(None, None, None)
            elif j % 4 == 3:
                for sl in range(2):
                    av_chunk(sl, j // 4)

        for j in range(NT):
            qk_j_h2(j)
            if j % 4 == 3:
                av_chunk(2, j // 4)

        bc_cm.__exit__(None, None, None)
        yq_cm.__exit__(None, None, None)
        st_cm.__exit__(None, None, None)

        # ---------------- proj: paired t-tiles ----------------
        with tc.tile_pool(name="ps_prj", bufs=4, space="PSUM") as ps_prj:
            for m in range(NT // 2):
                tts = (2 * m, 2 * m + 1)
                pjs = [ps_prj.tile([P, C], F32, tag="pj", name="pj") for _ in range(2)]
                for idx, tt in enumerate(tts):
                    for n0, nw in ((0, 512), (512, 256)):
                        nc.tensor.matmul(
                            pjs[idx][:, n0 : n0 + nw],
                            yT_a[:, tt * P : (tt + 1) * P],
                            wp_a[:, n0 : n0 + nw],
                            start=True,
                            stop=False,
                        )
                for idx, tt in enumerate(tts):
                    lo = idx * HD
                    for n0, nw in ((0, 512), (512, 256)):
                        nc.tensor.matmul(
                            pjs[idx][:, n0 : n0 + nw],
                            yT_b[lo : lo + HD, tt * P : (tt + 1) * P],
                            wp_b[lo : lo + HD, n0 : n0 + nw],
                            start=False,
                            stop=True,
                        )
                for idx, tt in enumerate(tts):
                    ot = out_pool.tile([P, C], BF16, tag="o")
                    nc.vector.tensor_copy(out=ot[:], in_=pjs[idx][:])
                    nc.sync.dma_start(out_d[tt * P : (tt + 1) * P, :], ot[:])


@functools.cache
def _build():
    nc = bacc.Bacc(
        "TRN2",
        target_bir_lowering=False,
        debug=False,
        enable_asserts=False,
        num_devices=8,
    )
    xt_d = nc.dram_tensor("xt", [C, T], BF16, kind="ExternalInput").ap()
    wqk_d = nc.dram_tensor("wqk", [C, QKW], BF16, kind="ExternalInput").ap()
    wv_d = nc.dram_tensor("wv", [C, VW], BF16, kind="ExternalInput").ap()
    wp_d = nc.dram_tensor("wp", [VW, C], BF16, kind="ExternalInput").ap()
    out_d = nc.dram_tensor("out", [T, C], BF16, kind="ExternalOutput").ap()
    with tile.TileContext(nc) as tc:
        _emit(nc, tc, xt_d, wqk_d, wv_d, wp_d, out_d)
    nc.compile()
    return nc


def kernel(x, mask, Wqkv, Wproj):
    global LAST_RESULT
    x = np.asarray(x, dtype=np.float32)
    Wqkv = np.asarray(Wqkv, dtype=np.float32)
    Wproj = np.asarray(Wproj, dtype=np.float32)

    xt_b = [
        np.ascontiguousarray(x[b].T).astype(BF16NP) for b in range(B)
    ]
    in_maps = []
    for c in range(8):
        b, g = divmod(c, 4)
        hs = [HPG * g + i for i in range(HPG)]

        def col(base, h):
            return Wqkv[:, base + HD * h : base + HD * h + HD]

        wqk = np.concatenate(
            [col(0, hs[0]), col(0, hs[1]), col(C, hs[0]), col(C, hs[1]),
             col(0, hs[2]), col(C, hs[2])],
            axis=1,
        )
        wv = np.concatenate([col(2 * C, h) for h in hs], axis=1)
        wp = Wproj[VW * g : VW * (g + 1), :]
        in_maps.append(
            {
                "xt": xt_b[b],
                "wqk": np.ascontiguousarray(wqk).astype(BF16NP),
                "wv": np.ascontiguousarray(wv).astype(BF16NP),
                "wp": np.ascontiguousarray(wp).astype(BF16NP),
            }
        )

    nc = _build()
    res = run_bass_kernel_spmd(nc, in_maps, core_ids=list(range(8)))
    LAST_RESULT = res
    out = np.empty((B, T, C), dtype=np.float32)
    for b in range(B):
        acc = res.results[4 * b]["out"].astype(np.float32)
        for g in range(1, 4):
            acc = acc + res.results[4 * b + g]["out"].astype(np.float32)
        out[b] = acc
    return out


if __name__ == "__main__":
    rng = np.random.default_rng(0)
    x = rng.standard_normal((B, T, C), dtype=np.float32)
    wqkv = rng.standard_normal((C, 3 * C), dtype=np.float32) / np.sqrt(C)
    wproj = rng.standard_normal((C, C), dtype=np.float32) / np.sqrt(C)
    o = kernel(x, None, wqkv, wproj)
    print(o.shape, o.dtype)
